# revision 1
# baseline (speedup 1.0000x reference)
"""Trainium2 Bass kernel for CodecLlamaCodecEmbedding (MoE-routed per-codebook MLP).

Strategy (expert-parallel): there are 8 codebooks and 8 NeuronCores. The host
sorts tokens by codebook (the MoE dispatch) and sends core k exactly the tokens
belonging to codebook k (padded to a 128-aligned capacity so the SPMD program
is static), already gathered from the embedding table and transposed to
feature-major [16, cap] layout, plus that codebook's projector weights.

Each core then runs the 2-layer projector entirely on-device:
  layer 1:  hT = gelu(W1.T @ eT + b1)   feature-major [2048, cap], fp32 exact
            erf GELU on ScalarE with the bias fused into the activation.
  layer 2:  out[tok, :] = hT.T @ W2 + b2, accumulated over 16 K-chunks in
            PSUM; kc is the outer loop so the 16 MB W2 load streams from HBM
            directly into the accumulation, and each hT chunk is loaded into
            the PE array once per 4 matmuls (2048 moving columns).
Matmul operands use dtype float32r: full-rate PE streaming (4x faster than
plain fp32) at ~1e-4 relative error (measured on HW, ~15x tighter than bf16).
b2 is added on VectorE during the PSUM->SBUF copy, then [128, 512] blocks are
DMAd to DRAM. The host scatters the 8 per-core outputs back to token order.
"""

import math
from contextlib import ExitStack

import numpy as np

import concourse.bacc as bacc
import concourse.tile as tile
from concourse import mybir
from concourse.bass_utils import run_bass_kernel_spmd

# Problem constants (hardcoded per the harness contract).
NUM_CODEBOOKS = 8
CODEBOOK_SIZE = 2048
D = 16        # codebook embedding dim
H = 2048      # hidden size
V = NUM_CODEBOOKS * CODEBOOK_SIZE  # embed table rows
N_CORES = 8

P = 128                  # SBUF partitions / tile edge
CAP = 2304               # default token capacity per core (mean 2048, sigma ~42)
KC = H // P              # 16 contraction chunks for layer 2
NFREE = 512              # matmul moving-operand free dim (1 PSUM bank of fp32)
NSPLIT = H // NFREE      # 4 output column chunks

F32 = mybir.dt.float32
# float32r streams through the PE at full rate (1 cycle/row vs 4 for plain
# fp32 when the moving dim >= 256) with ~1e-4 relative error (measured on HW;
# ~15x more accurate than bf16). Same 4-byte IEEE storage, numpy side is f32.
F32R = mybir.dt.float32r

TUNE = {
    "group": 4,     # token tiles per layer-1 batch (N = group*128 matmuls)
    "ht_bufs": 5,
    "ob_bufs": 4,
    "l1_bufs": 2,
    "l2_bufs": 6,
    "w2_split": 4,  # W2 chunk DMA granularity (finer = smoother streaming)
}


def _emit(ctx: ExitStack, tc: tile.TileContext, aps: dict, nt: int,
          act=mybir.ActivationFunctionType.Gelu, tune=None, mm_dt=F32R, mm_dt2=None):
    mm_dt2 = mm_dt if mm_dt2 is None else mm_dt2
    t = dict(TUNE)
    t.update(tune or {})
    group = t["group"]
    nc = tc.nc
    et_ap = aps["et"]        # [D, cap] f32r, pre-gathered transposed embeddings
    w1_ap = aps["w1"]        # [D, H]  f32r
    b1_ap = aps["b1"]        # [P, KC] f32, b1_ap[p, c] = b1[c*128 + p]
    w2_ap = aps["w2"]        # [H, H]  f32r
    b2_ap = aps["b2"]        # [P, H]  f32, b2 replicated across partitions
    out_ap = aps["out"]      # [cap, H] f32

    const = ctx.enter_context(tc.tile_pool(name="const", bufs=1))
    w2p = ctx.enter_context(tc.tile_pool(name="w2p", bufs=1))
    htp = ctx.enter_context(tc.tile_pool(name="htp", bufs=t["ht_bufs"]))
    op = ctx.enter_context(tc.tile_pool(name="op", bufs=t["ob_bufs"]))
    l1p = ctx.enter_context(tc.tile_pool(name="l1p", bufs=t["l1_bufs"], space="PSUM"))
    l2p = ctx.enter_context(tc.tile_pool(name="l2p", bufs=t["l2_bufs"], space="PSUM"))

    # Small inputs first so they clear the DMA engines before the W2 stream.
    w1_sb = const.tile([D, H], mm_dt)
    nc.sync.dma_start(w1_sb[:], w1_ap[:, :])
    b1_sb = const.tile([P, KC], F32)
    nc.sync.dma_start(b1_sb[:], b1_ap[:, :])
    # The whole embedding block is tiny (16 x cap f32); land it before W2.
    et_sb = const.tile([D, nt * P], mm_dt)
    nc.sync.dma_start(et_sb[:], et_ap[:, :])
    # b2 (1 MB) is not needed until the first PSUM drains, ~40us in.
    b2_sb = const.tile([P, H], F32)
    nc.sync.dma_start(b2_sb[:], b2_ap[:, :])

    # W2 resident in SBUF: chunk kc holds rows [kc*128, (kc+1)*128) of W2,
    # laid out at columns [kc*H, (kc+1)*H). Streamed in chunk order; layer 2
    # consumes chunks in the same order, so compute starts before the load
    # finishes.
    w2_sb = w2p.tile([P, KC * H], mm_dt2)
    wsplit = t.get("w2_split", 1)
    for kc in range(KC):
        for s in range(wsplit):
            c0, c1 = s * (H // wsplit), (s + 1) * (H // wsplit)
            nc.sync.dma_start(
                w2_sb[:, kc * H + c0:kc * H + c1],
                w2_ap[kc * P:(kc + 1) * P, c0:c1],
            )

    # Balanced groups of <=`group` tiles, as equal as possible, so every
    # layer-1 matmul keeps a moving dim >= 256 (f32r full-rate region).
    n_groups = -(-nt // group)
    base, extra = divmod(nt, n_groups)
    sizes = [base + (1 if g < extra else 0) for g in range(n_groups)]
    starts = [sum(sizes[:g]) for g in range(n_groups)]
    for g0, gsz in zip(starts, sizes):
        tts = list(range(g0, g0 + gsz))
        w = len(tts) * P

        eT = et_sb[:, g0 * P:g0 * P + w]

        # Layer 1: hT[h, tok] = gelu(W1[:, h] . eT[:, tok] + b1[h]), stored
        # feature-major: ht tile [128 (h in chunk), 16 chunks x 128 tokens].
        hts = [htp.tile([P, H], mm_dt2, tag="ht", name=f"ht_{tt}") for tt in tts]
        for hc in range(KC):
            ps1 = l1p.tile([P, group * P], F32, tag="l1")
            nc.tensor.matmul(
                ps1[:, :w],
                w1_sb[:, hc * P:(hc + 1) * P],
                eT,
                start=True,
                stop=True,
            )
            for j in range(len(tts)):
                nc.scalar.activation(
                    hts[j][:, hc * P:(hc + 1) * P],
                    ps1[:, j * P:(j + 1) * P],
                    act,
                    bias=b1_sb[:, hc:hc + 1],
                )

        # Layer 2: out[tok, n] = sum_kc hT[kc][:, tok].T @ W2[kc][:, n] + b2[n]
        # kc outer: one hT weight load feeds 4 matmuls, and the first tiles
        # start as soon as the first W2 chunks land.
        pair = t.get("l2_pair", 1)
        for j0 in range(0, len(tts), pair):
            js = list(range(j0, min(j0 + pair, len(tts))))
            pss = {
                (j, n): l2p.tile([P, NFREE], F32, tag="l2", name=f"ps2_{tts[j]}_{n}")
                for j in js for n in range(NSPLIT)
            }
            # kc-major across the tile pair: the PE instruction stream consumes
            # W2 chunks in arrival order instead of tile 1 queueing behind
            # tile 0's last chunk.
            for kc in range(KC):
                for j in js:
                    for n in range(NSPLIT):
                        nc.tensor.matmul(
                            pss[j, n][:],
                            hts[j][:, kc * P:(kc + 1) * P],
                            w2_sb[:, kc * H + n * NFREE: kc * H + (n + 1) * NFREE],
                            start=(kc == 0),
                            stop=(kc == KC - 1),
                        )
            for j in js:
                tt = tts[j]
                for n in range(NSPLIT):
                    ob = op.tile([P, NFREE], F32, tag="ob")
                    nc.vector.tensor_add(ob[:], pss[j, n][:], b2_sb[:, n * NFREE:(n + 1) * NFREE])
                    nc.sync.dma_start(
                        out_ap[tt * P:(tt + 1) * P, n * NFREE:(n + 1) * NFREE], ob[:]
                    )


def build_nc(cap=CAP, act=mybir.ActivationFunctionType.Gelu, tune=None, mm_dt=F32R, mm_dt2=None):
    mm_dt2 = mm_dt if mm_dt2 is None else mm_dt2
    assert cap % P == 0 and cap > 0
    nt = cap // P
    nc = bacc.Bacc("TRN2", target_bir_lowering=False, debug=False)
    aps = {
        "et": nc.dram_tensor("et", [D, cap], mm_dt, kind="ExternalInput").ap(),
        "w1": nc.dram_tensor("w1", [D, H], mm_dt, kind="ExternalInput").ap(),
        "b1": nc.dram_tensor("b1", [P, KC], F32, kind="ExternalInput").ap(),
        "w2": nc.dram_tensor("w2", [H, H], mm_dt2, kind="ExternalInput").ap(),
        "b2": nc.dram_tensor("b2", [P, H], F32, kind="ExternalInput").ap(),
        "out": nc.dram_tensor("out", [cap, H], F32, kind="ExternalOutput").ap(),
    }
    with tile.TileContext(nc) as tc:
        with ExitStack() as ctx:
            _emit(ctx, tc, aps, nt, act=act, tune=tune, mm_dt=mm_dt, mm_dt2=mm_dt2)
    nc.compile()
    return nc


_NC_CACHE = {}


def _get_nc(cap=CAP):
    if cap not in _NC_CACHE:
        _NC_CACHE[cap] = build_nc(cap)
    return _NC_CACHE[cap]


def _gelu_exact_np(x):
    try:
        from scipy.special import erf
    except ImportError:
        erf = np.vectorize(math.erf)
    return 0.5 * x * (1.0 + erf(x / np.sqrt(2.0).astype(x.dtype)))


def _route(ids_flat: np.ndarray):
    """Sort token positions by codebook. Returns per-codebook position lists."""
    cb = ids_flat // CODEBOOK_SIZE
    order = np.argsort(cb, kind="stable")
    counts = np.bincount(cb, minlength=NUM_CODEBOOKS)
    starts = np.concatenate([[0], np.cumsum(counts)])
    return [order[starts[k]:starts[k + 1]] for k in range(NUM_CODEBOOKS)], counts


MAX_DEV_CAP = 4096  # beyond this (a ~48-sigma skew) overflow tokens go to host


def pick_cap(counts):
    """Smallest multiple of 128 covering the max per-codebook load."""
    need = max(int(counts.max()), P)
    nt = -(-need // P)
    return min(nt * P, MAX_DEV_CAP)


def make_in_maps(ids_flat, embed_table, W1, b1, W2, b2, cap=CAP):
    positions, counts = _route(ids_flat)
    table = np.ascontiguousarray(embed_table, dtype=np.float32)
    in_maps = []
    for k in range(NUM_CODEBOOKS):
        pos_k = positions[k][:cap]
        idx_pad = np.zeros(cap, np.int64)  # padding points at table row 0
        idx_pad[:len(pos_k)] = ids_flat[pos_k]
        in_maps.append({
            "et": np.ascontiguousarray(table[idx_pad].T),
            "w1": np.ascontiguousarray(W1[k], dtype=np.float32),
            "b1": np.ascontiguousarray(np.asarray(b1[k], dtype=np.float32).reshape(KC, P).T),
            "w2": np.ascontiguousarray(W2[k], dtype=np.float32),
            "b2": np.ascontiguousarray(
                np.broadcast_to(np.asarray(b2[k], dtype=np.float32), (P, H))
            ),
        })
    return in_maps, positions, counts


def kernel(codec_input_ids, embed_table, W1, b1, W2, b2):
    codec_input_ids = np.asarray(codec_input_ids)
    embed_table = np.asarray(embed_table, dtype=np.float32)
    W1 = np.asarray(W1, dtype=np.float32)
    b1 = np.asarray(b1, dtype=np.float32)
    W2 = np.asarray(W2, dtype=np.float32)
    b2 = np.asarray(b2, dtype=np.float32)

    B, S = codec_input_ids.shape
    ids_flat = codec_input_ids.reshape(-1).astype(np.int64)

    _, counts = _route(ids_flat)
    cap = pick_cap(counts)
    in_maps, positions, counts = make_in_maps(
        ids_flat, embed_table, W1, b1, W2, b2, cap=cap
    )

    try:
        nc = _get_nc(cap)
        results = run_bass_kernel_spmd(nc, in_maps, list(range(N_CORES))).results
    except Exception as e:  # device/compile fault: stay correct via host math
        import sys
        print(f"kernel: device path failed ({e!r}); host fallback", file=sys.stderr)
        results = None

    out_flat = np.zeros((B * S, H), np.float32)
    for k in range(NUM_CODEBOOKS):
        pos_k = positions[k]
        n_dev = min(len(pos_k), cap) if results is not None else 0
        if n_dev:
            out_flat[pos_k[:n_dev]] = results[k]["out"][:n_dev]
        if len(pos_k) > n_dev:
            # Overflow beyond the compiled capacity (never happens for the
            # reference input distribution) or device-fault fallback:
            # compute exactly on host.
            pos_of = pos_k[n_dev:]
            e = embed_table[ids_flat[pos_of]]
            h = _gelu_exact_np(e @ W1[k] + b1[k])
            out_flat[pos_of] = h @ W2[k] + b2[k]

    return out_flat.reshape(B, S, H)



# revision 35
# speedup vs baseline: 106.5807x; 106.5807x over previous
"""Trainium2 Bass kernel for CodecLlamaCodecEmbedding (MoE-routed per-codebook MLP).

Strategy (expert-parallel): there are 8 codebooks and 8 NeuronCores. The host
sorts tokens by codebook (the MoE dispatch) and sends core k exactly the tokens
belonging to codebook k (padded to a 128-aligned capacity so the SPMD program
is static), already gathered from the embedding table and transposed to
feature-major [16, cap] layout, plus that codebook's projector weights.

Each core then runs the 2-layer projector entirely on-device:
  layer 1:  hT = gelu(W1.T @ eT + b1)   feature-major [2048, cap], fp32 exact
            erf GELU on ScalarE with the bias fused into the activation.
  layer 2:  out[tok, :] = hT.T @ W2 + b2, accumulated over 16 K-chunks in
            PSUM; kc is the outer loop so the 8 MB W2 load streams from HBM
            directly into the accumulation, and each hT chunk is loaded into
            the PE array once per 4 matmuls (2048 moving columns).
Matmul operands use bfloat16: the PE streams bf16 at full rate AND the
stationary-weight load pipelines with in-flight matmuls (FWL + background
weight buffer), unlike float32r whose self-loading InstMatmult serializes a
~53 ns weight load into every matmul (~58 us over the kernel, measured).
bf16 end-to-end relative error vs the fp32 reference is ~2e-3 (measured on
the actual fixed-seed inputs), an order of magnitude inside the 2e-2 budget.
b2 is added on VectorE during the PSUM->SBUF copy, then [128, 512] blocks are
DMAd to DRAM. The host scatters the 8 per-core outputs back to token order.
"""

import math
from contextlib import ExitStack

import numpy as np

import concourse.bacc as bacc
import concourse.tile as tile
from concourse import mybir
from concourse.bass_utils import run_bass_kernel_spmd

# Problem constants (hardcoded per the harness contract).
NUM_CODEBOOKS = 8
CODEBOOK_SIZE = 2048
D = 16        # codebook embedding dim
H = 2048      # hidden size
V = NUM_CODEBOOKS * CODEBOOK_SIZE  # embed table rows
N_CORES = 8

P = 128                  # SBUF partitions / tile edge
CAP = 2304               # default token capacity per core (mean 2048, sigma ~42)
KC = H // P              # 16 contraction chunks for layer 2
NFREE = 512              # matmul moving-operand free dim (1 PSUM bank of fp32)
NSPLIT = H // NFREE      # 4 output column chunks

F32 = mybir.dt.float32
BF16 = mybir.dt.bfloat16

TUNE = {
    "group": 4,     # token tiles per layer-1 batch (N = group*128 matmuls)
    "ob_bufs": 4,
    "l1_bufs": 4,
    "l2_bufs": 4,
    "burst": 1,     # L1 fill units per interleave slot (a unit is already
                    # `row_pack` concurrent matmuls)
    "w2_split": 2,  # W2 chunk DMA granularity (finer = smoother streaming)
    # Tiles >= this index run layer 2 n-major: each PSUM bank finishes its
    # 16-chunk accumulation early and drains while the next bank computes,
    # so bank reuse never stalls the PE and the last tile has no drain tail.
    # Earlier tiles run kc-major to consume the W2 stream in arrival order.
    "n_major_from": 3,
    # L1 groups emitted onto the PE queue before any L2 work; remaining
    # groups are interleaved one per early L2 tile. The 8 MB W2 stream lands
    # at ~330 GB/s (~28 us) while one tile's L2 consumes it at ~600 GB/s
    # equivalent, so L1 work must fill the arrival deficit or the PE stalls.
    "l1_front": 1,
    "out_bf16": 1,  # write the output in bf16 (halves drain DMA; ~2e-3 rel)
    # Layer 1 contracts over only D=16 of 128 PE rows; packing 4 chunk
    # matmuls into disjoint 32-row strips (tile_position) runs them
    # concurrently, cutting L1 PE time ~4x.
    "row_pack": 4,
    # Matmuls on garbage SBUF right after the preamble: they warm the PE
    # clock gate (HAM) during the otherwise-idle wait for the first input
    # DMAs, so real matmuls start at 2.4 GHz instead of 1.2.
    "warm_mms": 12,
}


def _emit(ctx: ExitStack, tc: tile.TileContext, aps: dict, nt: int,
          act=mybir.ActivationFunctionType.Gelu, tune=None, mm_dt=BF16, mm_dt2=None):
    mm_dt2 = mm_dt if mm_dt2 is None else mm_dt2
    t = dict(TUNE)
    t.update(tune or {})
    group = t["group"]
    nc = tc.nc
    et_ap = aps["et"]        # [D, cap] bf16, pre-gathered transposed embeddings
    w1_ap = aps["w1"]        # [D, H]  bf16
    b1_ap = aps["b1"]        # [P, KC] f32, b1_ap[p, c] = b1[c*128 + p]
    w2_ap = aps["w2"]        # [H, H]  bf16
    b2_ap = aps["b2"]        # [P, H]  f32, b2 replicated across partitions
    out_ap = aps["out"]      # [cap, H] f32

    const = ctx.enter_context(tc.tile_pool(name="const", bufs=1))
    w2p = ctx.enter_context(tc.tile_pool(name="w2p", bufs=1))
    htp = ctx.enter_context(tc.tile_pool(name="htp", bufs=-(-nt // group)))
    op = ctx.enter_context(tc.tile_pool(name="op", bufs=t["ob_bufs"]))
    l1p = ctx.enter_context(tc.tile_pool(name="l1p", bufs=t["l1_bufs"], space="PSUM"))
    l2p = ctx.enter_context(tc.tile_pool(name="l2p", bufs=t["l2_bufs"], space="PSUM"))

    rp = t.get("row_pack", 0) or 1
    assert KC % rp == 0 and rp in (1, 2, 4)

    # PE warm-up on garbage SBUF (no input deps -> runs during the preamble
    # tail / first DMA waits).
    if t.get("warm_mms"):
        warm = const.tile([P, NFREE], mm_dt)
        nc.gpsimd.memset(warm[:], 0)
        wps = l1p.tile([P, group * P], F32, tag="l1", name="warm")
        for _ in range(t["warm_mms"]):
            nc.tensor.matmul(wps[:, :NFREE], warm[:, :P], warm[:],
                             start=True, stop=True)

    # Small inputs first so they clear the DMA engines before the W2 stream.
    # The host ships w1/et pre-replicated into `rp` 32-partition strips (for
    # row-packed layer-1 matmuls) so each lands in a single DMA — issuing
    # per-strip DMAs here would serialize ~5 us of descriptors on sync and
    # push layer 1 past the HAM re-throttle window (measured).
    # et rides gpsimd while w1+b1 ride sync, so layer 1's inputs are the
    # first descriptors on BOTH queues and complete before the W2 stream
    # saturates HBM (issued behind them, below).
    prows = 32 * rp if rp > 1 else D
    w1_sb = const.tile([prows, H], mm_dt)
    nc.sync.dma_start(w1_sb[:], w1_ap[:, :])
    et_sb = const.tile([prows, nt * P], mm_dt)
    nc.gpsimd.dma_start(et_sb[:], et_ap[:, :])
    b1_sb = const.tile([P, KC], F32)
    nc.sync.dma_start(b1_sb[:], b1_ap[:, :])
    b2_sb = const.tile([P, H], F32)

    # W2 resident in SBUF: chunk kc holds rows [kc*128, (kc+1)*128) of W2,
    # laid out at columns [kc*H, (kc+1)*H). Streamed in chunk order; layer 2
    # consumes chunks in the same order, so compute starts before the load
    # finishes. Each dma_start costs ~600 ns on its issuing engine's queue,
    # so the descriptors are spread across otherwise-idle engine queues —
    # serialized on sync alone, the last chunks would not even be issued
    # until ~45 us in (measured).
    # Scalar must stay off this list: DMA issues there push the GELU
    # ACT_TABLE_LOAD (and so every layer-1 drain) tens of us out. GpSimd
    # must not run any library custom-op (a LOAD_LIB blocks its queue ~14 us).
    w2_sb = w2p.tile([P, KC * H], mm_dt2)
    wsplit = t.get("w2_split", 1)
    dma_engs = [nc.gpsimd, nc.sync]
    di = 0
    for kc in range(KC):
        for s in range(wsplit):
            c0, c1 = s * (H // wsplit), (s + 1) * (H // wsplit)
            dma_engs[di % len(dma_engs)].dma_start(
                w2_sb[:, kc * H + c0:kc * H + c1],
                w2_ap[kc * P:(kc + 1) * P, c0:c1],
            )
            di += 1

    # b2 (1 MB) is only needed at the first PSUM drain ~25 us in; it queues
    # behind the W2 stream so it never steals early HBM bandwidth.
    nc.gpsimd.dma_start(b2_sb[:], b2_ap[:, :])

    # Balanced groups of <=`group` tiles, as equal as possible, so every
    # layer-1 matmul keeps a moving dim >= 256.
    n_groups = -(-nt // group)
    base, extra = divmod(nt, n_groups)
    sizes = [base + (1 if g < extra else 0) for g in range(n_groups)]
    starts = [sum(sizes[:g]) for g in range(n_groups)]
    # hts[tt] -> (group ht tile [P, gsz, H], j index within group)
    hts = [None] * nt
    out_dt = BF16 if t.get("out_bf16") else F32

    def l1_fills(g):
        """Yield layer-1 fill units (`rp` row-packed matmuls + merged
        activations each)."""
        g0, gsz = starts[g], sizes[g]
        w = gsz * P
        # Layer 1: hT[h, tok] = gelu(W1[:, h] . eT[:, tok] + b1[h]), stored
        # feature-major: htg[p, j, hc*128 + tok] for tile g0+j. One merged
        # [128, gsz*128] activation per fill keeps ScalarE off the critical
        # path (4 separate 128-col ACTIVATEs pay the ~260 ns setup 4x).
        htg = htp.tile([P, gsz, H], mm_dt2, tag="ht", name=f"ht_g{g}")
        for j in range(gsz):
            hts[g0 + j] = (htg, j)
        for hq in range(0, KC, rp):
            def fill(hq=hq):
                # rp concurrent matmuls in disjoint 32-row PE strips; 2D PSUM
                # out APs (a 3D matmul out drops off walrus's fast path:
                # ~600 ns vs ~380 ns per matmul, measured).
                pss = [l1p.tile([P, group * P], F32, tag="l1",
                                name=f"ps1_{g0}_{hq}_{i}")
                       for i in range(rp)]
                for i in range(rp):
                    hc = hq + i
                    off = 32 * i if rp > 1 else 0
                    nc.tensor.matmul(
                        pss[i][:, :w],
                        w1_sb[off:off + D, hc * P:(hc + 1) * P],
                        et_sb[off:off + D, g0 * P:g0 * P + w],
                        start=True,
                        stop=True,
                        tile_position=(off, 0),
                    )
                for i in range(rp):
                    hc = hq + i
                    nc.scalar.activation(
                        htg[:, :, hc * P:(hc + 1) * P],
                        pss[i][:, :w],
                        act,
                        bias=b1_sb[:, hc:hc + 1],
                    )
            yield fill

    def emit_l2(tt, interleave=None):
        # Layer 2: out[tok, n] = sum_kc hT[kc][:, tok].T @ W2[kc][:, n] + b2[n]
        # Walrus emits an LDWEIGHTS per matmul either way (pipelined through
        # the PE's 64-deep reorder window), so the loop order is free:
        #  - early tiles go kc-major so the PE consumes W2 chunks in DMA
        #    arrival order while the 8 MB stream is still landing;
        #  - later tiles go n-major so each PSUM bank finishes early and its
        #    bias-add + store overlap the next bank's accumulation (no bank-
        #    reuse stall, no drain tail after the final matmul).
        def drain(n, ps):
            ob = op.tile([P, NFREE], out_dt, tag="ob")
            nc.vector.tensor_add(ob[:], ps[:], b2_sb[:, n * NFREE:(n + 1) * NFREE])
            nc.sync.dma_start(
                out_ap[tt * P:(tt + 1) * P, n * NFREE:(n + 1) * NFREE], ob[:]
            )

        def burst():
            if interleave:
                for _ in range(t["burst"]):
                    f = next(interleave, None)
                    if f:
                        f()

        htg, j = hts[tt]
        if tt >= t.get("n_major_from", 0):
            for n in range(NSPLIT):
                ps = l2p.tile([P, NFREE], F32, tag="l2", name=f"ps2_{tt}_{n}")
                for kc in range(KC):
                    nc.tensor.matmul(
                        ps[:],
                        htg[:, j, kc * P:(kc + 1) * P],
                        w2_sb[:, kc * H + n * NFREE: kc * H + (n + 1) * NFREE],
                        start=(kc == 0),
                        stop=(kc == KC - 1),
                    )
                drain(n, ps)
                burst()
        else:
            pss = [l2p.tile([P, NFREE], F32, tag="l2", name=f"ps2_{tt}_{n}")
                   for n in range(NSPLIT)]
            for kc in range(KC):
                for n in range(NSPLIT):
                    nc.tensor.matmul(
                        pss[n][:],
                        htg[:, j, kc * P:(kc + 1) * P],
                        w2_sb[:, kc * H + n * NFREE: kc * H + (n + 1) * NFREE],
                        start=(kc == 0),
                        stop=(kc == KC - 1),
                    )
                if kc % 4 == 3:
                    burst()
            for n in range(NSPLIT):
                drain(n, pss[n])

    # Schedule: L1(g0) runs alone up front; the remaining groups' fill units
    # are interleaved one-per-W2-chunk into the early L2 tiles, so the PE
    # has W2-independent work exactly while the 8 MB W2 stream is landing
    # (its ~330 GB/s arrival is slower than one tile's ~600 GB/s-equivalent
    # consumption). Later tiles run pure L2 with the stream fully resident.
    def remaining_fills():
        for g in range(t["l1_front"], n_groups):
            yield from l1_fills(g)

    for g in range(min(t["l1_front"], n_groups)):
        for f in l1_fills(g):
            f()
    fills = remaining_fills()
    for tt in range(nt):
        assert hts[tt] is not None, "L1 must precede L2 for each tile"
        emit_l2(tt, interleave=fills)
    for f in fills:  # tiny nt edge case: flush any unemitted fills
        f()


def build_nc(cap=CAP, act=mybir.ActivationFunctionType.Gelu, tune=None, mm_dt=BF16, mm_dt2=None):
    mm_dt2 = mm_dt if mm_dt2 is None else mm_dt2
    assert cap % P == 0 and cap > 0
    nt = cap // P
    t = dict(TUNE)
    t.update(tune or {})
    out_dt = BF16 if t.get("out_bf16") else F32
    rp = t.get("row_pack", 0) or 1
    prows = 32 * rp if rp > 1 else D
    nc = bacc.Bacc("TRN2", target_bir_lowering=False, debug=False)
    aps = {
        "et": nc.dram_tensor("et", [prows, cap], mm_dt, kind="ExternalInput").ap(),
        "w1": nc.dram_tensor("w1", [prows, H], mm_dt, kind="ExternalInput").ap(),
        "b1": nc.dram_tensor("b1", [P, KC], F32, kind="ExternalInput").ap(),
        "w2": nc.dram_tensor("w2", [H, H], mm_dt2, kind="ExternalInput").ap(),
        "b2": nc.dram_tensor("b2", [P, H], F32, kind="ExternalInput").ap(),
        "out": nc.dram_tensor("out", [cap, H], out_dt, kind="ExternalOutput").ap(),
    }
    with tile.TileContext(nc) as tc:
        with ExitStack() as ctx:
            _emit(ctx, tc, aps, nt, act=act, tune=tune, mm_dt=mm_dt, mm_dt2=mm_dt2)
    nc.compile()
    return nc


_NC_CACHE = {}


def _get_nc(cap=CAP):
    if cap not in _NC_CACHE:
        _NC_CACHE[cap] = build_nc(cap)
    return _NC_CACHE[cap]


def _np_dt(mm_dt):
    return mybir.dt.np(mm_dt)


def _gelu_exact_np(x):
    try:
        from scipy.special import erf
    except ImportError:
        erf = np.vectorize(math.erf)
    return 0.5 * x * (1.0 + erf(x / np.sqrt(2.0).astype(x.dtype)))


def _route(ids_flat: np.ndarray):
    """Sort token positions by codebook. Returns per-codebook position lists."""
    cb = ids_flat // CODEBOOK_SIZE
    order = np.argsort(cb, kind="stable")
    counts = np.bincount(cb, minlength=NUM_CODEBOOKS)
    starts = np.concatenate([[0], np.cumsum(counts)])
    return [order[starts[k]:starts[k + 1]] for k in range(NUM_CODEBOOKS)], counts


# Beyond this (a ~24-sigma skew for the reference distribution), overflow
# tokens go to host math; larger caps would also overflow the ht-tile SBUF
# budget (the htp pool scales with cap).
MAX_DEV_CAP = 3072


def pick_cap(counts):
    """Smallest multiple of 128 covering the max per-codebook load."""
    need = max(int(counts.max()), P)
    nt = -(-need // P)
    return min(nt * P, MAX_DEV_CAP)


def _strip_rep(a, rp):
    """Replicate [D, X] into rp 32-partition strips: rows 32*i+p = a[p]."""
    if rp <= 1:
        return np.ascontiguousarray(a)
    out = np.zeros((32 * rp, a.shape[1]), a.dtype)
    for i in range(rp):
        out[32 * i:32 * i + D] = a
    return out


def make_in_maps(ids_flat, embed_table, W1, b1, W2, b2, cap=CAP, mm_dt=BF16):
    positions, counts = _route(ids_flat)
    table = np.ascontiguousarray(embed_table, dtype=np.float32)
    np_mm = _np_dt(mm_dt)
    rp = TUNE.get("row_pack", 0) or 1
    in_maps = []
    for k in range(NUM_CODEBOOKS):
        pos_k = positions[k][:cap]
        idx_pad = np.zeros(cap, np.int64)  # padding points at table row 0
        idx_pad[:len(pos_k)] = ids_flat[pos_k]
        in_maps.append({
            "et": _strip_rep(np.ascontiguousarray(table[idx_pad].T).astype(np_mm), rp),
            "w1": _strip_rep(np.ascontiguousarray(W1[k], dtype=np.float32).astype(np_mm), rp),
            "b1": np.ascontiguousarray(np.asarray(b1[k], dtype=np.float32).reshape(KC, P).T),
            "w2": np.ascontiguousarray(W2[k], dtype=np.float32).astype(np_mm),
            "b2": np.ascontiguousarray(
                np.broadcast_to(np.asarray(b2[k], dtype=np.float32), (P, H))
            ),
        })
    return in_maps, positions, counts


def kernel(codec_input_ids, embed_table, W1, b1, W2, b2):
    codec_input_ids = np.asarray(codec_input_ids)
    embed_table = np.asarray(embed_table, dtype=np.float32)
    W1 = np.asarray(W1, dtype=np.float32)
    b1 = np.asarray(b1, dtype=np.float32)
    W2 = np.asarray(W2, dtype=np.float32)
    b2 = np.asarray(b2, dtype=np.float32)

    B, S = codec_input_ids.shape
    ids_flat = codec_input_ids.reshape(-1).astype(np.int64)

    _, counts = _route(ids_flat)
    cap = pick_cap(counts)
    in_maps, positions, counts = make_in_maps(
        ids_flat, embed_table, W1, b1, W2, b2, cap=cap
    )

    try:
        nc = _get_nc(cap)
        results = run_bass_kernel_spmd(nc, in_maps, list(range(N_CORES))).results
    except Exception as e:  # device/compile fault: stay correct via host math
        import sys
        print(f"kernel: device path failed ({e!r}); host fallback", file=sys.stderr)
        results = None

    out_flat = np.zeros((B * S, H), np.float32)
    for k in range(NUM_CODEBOOKS):
        pos_k = positions[k]
        n_dev = min(len(pos_k), cap) if results is not None else 0
        if n_dev:
            out_flat[pos_k[:n_dev]] = results[k]["out"][:n_dev].astype(np.float32)
        if len(pos_k) > n_dev:
            # Overflow beyond the compiled capacity (never happens for the
            # reference input distribution) or device-fault fallback:
            # compute exactly on host.
            pos_of = pos_k[n_dev:]
            e = embed_table[ids_flat[pos_of]]
            h = _gelu_exact_np(e @ W1[k] + b1[k])
            out_flat[pos_of] = h @ W2[k] + b2[k]

    return out_flat.reshape(B, S, H)
